# revision 87
# baseline (speedup 1.0000x reference)
"""Trainium2 Bass kernel for nn_CNN_Comp_29240137351522 (dense_cnn), v2.

Math:  y = |IFFT_N( FFT_N(x)^2 * C )|,  C = FFT_N(w0)^2 * FFT_N(wl) / N
with N = 2304 (= 128*18).  2304 >= 2303 covers the autoconv h*h exactly, and
the final circular conv aliases y[n+2304] only onto n < 255, which the center
crop [255:2303) discards, so the cropped result is exact.

Device decomposition per core (data-parallel over batch, S = 512 samples):
  n = n2*128 + n1 (n2 in [0,18), x nonzero for n2 < 8),  k = 18*k1 + k2
  F1 (contract n2, block-diag over j = n1 mod 16, twiddle folded, bf16);
      the weight-DFT, C*N and G builds are interleaved into the F1 loop
  pivot-C (DMA)   -> Abig[n1, (k2, plane, s)]; evicts alternate ACT/DVE and
      each pivot ships from its producer queue (ACT) or SP, so nothing
      head-of-line blocks the in-order DMA paths; bulky late consts ride
      the Pool SWDGE path instead of HWDGE
  F3 (contract n1, shared W128, bf16) -> X[k1, (k2, s)] in PSUM
  square: one ACT copy X->SBUF, squares/Zr on DVE 2x, Zi = Xr*Xi on Pool
      (the factor 2 is folded into the G2r/Gn2 planes of G)
  I1 (contract k1, G = C-row-scaled inverse DFT built on device, bf16)
  pivot-D (Pool SWDGE DMA) -> u2[(jp, k2), (plane, s)], 19 n1-groups of 7
  I2 (contract k2, block-diag over jp, bf16) + |.|^2 + sqrt, with sqrt and
      the store batched per pair of groups -> yraw (bf16)
Host does data movement only: batch shard, x permutation into the F1-ready
layout, packing of weight vectors, and the output unscramble.
"""

import os

import numpy as np
import ml_dtypes

DEBUG_ABIG = bool(os.environ.get("KDEBUG_ABIG"))

import concourse.bass as bass
import concourse.bacc as bacc
import concourse.mybir as mybir
from concourse.tile import TileContext
from concourse.bass_utils import run_bass_kernel_spmd

# ---------------- static problem config ----------------
B, NX = 4096, 1024
K0, KL = 129, 257
N = 2304
N1, N2 = 128, 18
NCORES = 8
S = B // NCORES              # 512 samples per core, single chunk
CROP0 = 255
CLASS_NUM = 2048
K2SPLIT = ((0, 8), (8, 16), (16, 18))     # F1 column splits (k2-major)
F1COLS = 288                               # 18*16 cols per g
JBLK = ((0, 7), (7, 14), (14, 16))        # I2 j-blocks per g
I2COLS = 272                               # 16*17 cols per g
YROWS = 119                                # max I2 out rows (7*17)

f32 = mybir.dt.float32
f32r = mybir.dt.float32r
bf16 = mybir.dt.bfloat16
AO = mybir.AluOpType
AF = mybir.ActivationFunctionType

BF = ml_dtypes.bfloat16


def _w(num, den):
    return np.exp(-2j * np.pi * np.asarray(num, np.float64) / den)


# ---------------- host-side constant arrays ----------------
def _build_consts():
    c = {}
    n1g = np.arange(N1)
    k1g = np.arange(N1)
    k2g = np.arange(N2)

    # F1 lhsT [128, 8*288]: row p = 8j + n2 ; col g*288 + sbase + k2sub*16 + j
    # value W18^{n2 k2} * W2304^{(16g+j) k2}
    f1 = np.zeros((128, 8 * F1COLS), np.complex128)
    for g in range(8):
        for (k2lo, k2hi), sbase in zip(K2SPLIT, (0, 128, 256)):
            nk = k2hi - k2lo
            for k2 in range(k2lo, k2hi):
                for j in range(16):
                    n1 = 16 * g + j
                    col = g * F1COLS + sbase + j * nk + (k2 - k2lo)
                    vals = _w(np.arange(8) * k2, N2) * _w(n1 * k2, N)
                    f1[8 * j : 8 * j + 8, col] = vals
    c["cf1"] = np.concatenate(
        [f1.real, f1.imag, -f1.imag], axis=1).astype(BF)   # [128, 3*2304]

    # F3 lhsT (shared): W128[n1,k1], bf16 + f32 copy for the weight-DFT mms
    w3 = _w(np.outer(n1g, k1g), N1)
    w3cat = np.concatenate([w3.real, w3.imag, -w3.imag], axis=1)
    c["cw3"] = w3cat.astype(BF)                            # [128, 384]
    c["cw3f"] = w3cat.astype(np.float32)                   # [128, 384]

    # inverse-DFT base tiled over k2, divided by N (folds the 1/N of C):
    # cwiB[:, v*2304 + k2*128 + p] = {Re,Im}(W128^{-k1 p}) / N
    wi = _w(-np.outer(k1g, n1g), N1) / N
    blk = np.concatenate([np.tile(wi.real, (1, N2)), np.tile(wi.imag, (1, N2))], axis=1)
    c["cwiB"] = blk.astype(BF)                             # [128, 2*2304]

    # I2 lhsT [128, 3*2176]: 19 n1-groups (18 of size 7 + 1 of size 2).
    # Per group t: partition p = jp*18 + k2, col TB[t] + jp*17 + (q-1);
    # value W18^{-q k2} * W2304^{-(7t+jp) k2}, q in [1,18)
    i2 = np.zeros((128, 2176), np.complex128)
    qg = np.arange(1, 18)
    for t in range(19):
        sz = 7 if t < 18 else 2
        for jp in range(sz):
            n1 = 7 * t + jp
            blkv = _w(-np.outer(k2g, qg), N2) * _w(-n1 * k2g, N)[:, None]
            rows = slice(jp * 18, jp * 18 + 18)
            cols = slice(t * 119 + jp * 17, t * 119 + (jp + 1) * 17)
            i2[rows, cols] = blkv
    c["ci2"] = np.concatenate(
        [i2.real, i2.imag, -i2.imag], axis=1).astype(BF)   # [128, 3*2176]

    # weight-DFT rhs constants (f32), packed into one [128, 272] tensor:
    # cols 0:18 ct1r | 18:36 ct1i | 36:54 ct2r | 54:72 ct2i
    # row0 cols 72:90 te1r | 90:108 te1i | 108:126 te2r | 126:144 te2i
    # row0 cols 144:272 ones (128)
    nh = np.arange(128)
    sm = np.zeros((128, 272), np.float32)
    t1 = _w(np.outer(nh, k2g), N)
    sm[:, 0:18] = t1.real
    sm[:, 18:36] = t1.imag
    t2 = _w(np.outer(nh, k2g), N) * _w(k2g, N2)[None, :]
    sm[:, 36:54] = t2.real
    sm[:, 54:72] = t2.imag
    te1 = _w(k2g, N2)
    sm[0, 72:90] = te1.real
    sm[0, 90:108] = te1.imag
    te2 = _w(k2g, 9)
    sm[0, 108:126] = te2.real
    sm[0, 126:144] = te2.imag
    sm[0, 144:272] = 1.0
    c["csm"] = sm

    return c


CONSTS = _build_consts()


# ---------------- bass kernel builder ----------------
def build_nc():
    nc = bacc.Bacc("TRN2", target_bir_lowering=False, debug=False, num_devices=NCORES)

    d = {}
    d["xt"] = nc.dram_tensor("xt", [128, 8192], bf16, kind="ExternalInput")
    d["wpack"] = nc.dram_tensor("wpack", [128, 10], f32, kind="ExternalInput")
    cdt = {"cw3f": f32, "csm": f32}
    for nm, arr in CONSTS.items():
        d[nm] = nc.dram_tensor(nm, list(arr.shape), cdt.get(nm, bf16), kind="ExternalInput")
    yraw = nc.dram_tensor("yraw", [YROWS, 19 * 512], bf16, kind="ExternalOutput")
    adbg = (nc.dram_tensor("adbg", [128, 18432], bf16, kind="ExternalOutput")
            if DEBUG_ABIG else None)

    with TileContext(nc) as tc:
        with (
            tc.tile_pool(name="cp", bufs=1) as cp,          # persistent consts
            tc.tile_pool(name="bp", bufs=1) as bp,          # Abig / Ubig / G
            tc.tile_pool(name="sp", bufs=2) as sp,          # rotating stage tiles
            tc.tile_pool(name="gp", bufs=2) as gp,          # G-build temporaries
            tc.tile_pool(name="stp", bufs=6) as stp,        # pivot-C staging
            tc.tile_pool(name="xp2", bufs=2) as xp2,        # xi/y copies
            tc.tile_pool(name="up", bufs=3) as up,          # u2 tiles
            tc.tile_pool(name="yp", bufs=2) as yp,          # yy tiles
            tc.tile_pool(name="zp", bufs=4) as zp,          # z tiles
            tc.tile_pool(name="tp", bufs=1) as tp,          # small f32 tmps
            tc.tile_pool(name="psa", bufs=2, space="PSUM") as psa,
        ):
            # ---- const + input DMAs (sync engine; just-in-time ordering:
            # inputs are chunked per-2u and interleaved with the pivot DMAs
            # inside the F1 loop so pivots aren't queued behind the whole
            # input storm on the in-order SP/DMA path) ----
            cf1 = cp.tile([128, 3 * 2304], bf16, tag="cf1")
            xt = cp.tile([128, 8192], bf16, tag="xt")

            def in_chunk(q):               # xt cols for g = q (old layout)
                gs = slice(q * 1024, (q + 1) * 1024)
                nc.sync.dma_start(out=xt[:, gs], in_=d["xt"][:, gs])

            nc.sync.dma_start(out=cf1[:, 0:2304], in_=d["cf1"][:, 0:2304])
            in_chunk(0)
            wpk = cp.tile([128, 10], f32, tag="wpack")
            nc.sync.dma_start(out=wpk[:], in_=d["wpack"][:, :])
            csm = cp.tile([128, 272], f32, tag="csm")
            nc.sync.dma_start(out=csm[:], in_=d["csm"][:, :])
            nc.sync.dma_start(out=cf1[:, 4608:6912], in_=d["cf1"][:, 4608:6912])
            in_chunk(1)
            nc.sync.dma_start(out=cf1[:, 2304:4608], in_=d["cf1"][:, 2304:4608])
            cw3f = cp.tile([128, 384], f32, tag="cw3f")
            nc.sync.dma_start(out=cw3f[:], in_=d["cw3f"][:, :])
            cw3 = cp.tile([128, 384], bf16, tag="cw3")
            cwiB = cp.tile([128, 2 * 2304], bf16, tag="cwiB")
            ci2 = cp.tile([128, 3 * 2176], bf16, tag="ci2")

            # ---- weight DFT rhs prep (DVE; needs only wpack/csm) ----
            def cplx_rhs(rows, tr, ti, cr, ci, outr, outi):
                # (cr + i ci) * (tr + i ti); cr/ci are [rows,1] scalar APs
                t = tp.tile([128, 18], f32, tag="wtmp")
                nc.vector.tensor_scalar(t[:rows, :], ti, ci, None, AO.mult)
                nc.vector.scalar_tensor_tensor(outr, tr, cr, t[:rows, :], AO.mult, AO.subtract)
                t2 = tp.tile([128, 18], f32, tag="wtmp2")
                nc.vector.tensor_scalar(t2[:rows, :], tr, ci, None, AO.mult)
                nc.vector.scalar_tensor_tensor(outi, ti, cr, t2[:rows, :], AO.mult, AO.add)

            rhs0 = tp.tile([128, 36], f32, tag="rhs0")
            cplx_rhs(128, csm[:, 0:18], csm[:, 18:36], wpk[:, 0:1], wpk[:, 1:2],
                     rhs0[:, 0:18], rhs0[:, 18:36])
            tl0 = tp.tile([1, 36], f32, tag="tl0")
            cplx_rhs(1, csm[0:1, 72:90], csm[0:1, 90:108], wpk[0:1, 6:7], wpk[0:1, 7:8],
                     tl0[:, 0:18], tl0[:, 18:36])
            rhs1 = tp.tile([128, 36], f32, tag="rhs1")
            cplx_rhs(128, csm[:, 0:18], csm[:, 18:36], wpk[:, 2:3], wpk[:, 3:4],
                     rhs1[:, 0:18], rhs1[:, 18:36])
            rhs2 = tp.tile([128, 36], f32, tag="rhs2")
            cplx_rhs(128, csm[:, 36:54], csm[:, 54:72], wpk[:, 4:5], wpk[:, 5:6],
                     rhs2[:, 0:18], rhs2[:, 18:36])
            tl2 = tp.tile([1, 36], f32, tag="tl2")
            cplx_rhs(1, csm[0:1, 108:126], csm[0:1, 126:144], wpk[0:1, 8:9], wpk[0:1, 9:10],
                     tl2[:, 0:18], tl2[:, 18:36])

            w3fr = cw3f[:, 0:128]
            w3fi = cw3f[:, 128:256]
            w3fn = cw3f[:, 256:384]
            onesf = csm[0:1, 144:272]

            def emit_w0ps():
                w0ps = psa.tile([128, 36], f32, tag="pX")
                nc.tensor.matmul(w0ps[:, 0:18], w3fr, rhs0[:, 0:18], start=True, stop=False)
                nc.tensor.matmul(w0ps[:, 0:18], w3fn, rhs0[:, 18:36], start=False, stop=False)
                nc.tensor.matmul(w0ps[:, 0:18], onesf, tl0[:, 0:18], start=False, stop=True)
                nc.tensor.matmul(w0ps[:, 18:36], w3fi, rhs0[:, 0:18], start=True, stop=False)
                nc.tensor.matmul(w0ps[:, 18:36], w3fr, rhs0[:, 18:36], start=False, stop=False)
                nc.tensor.matmul(w0ps[:, 18:36], onesf, tl0[:, 18:36], start=False, stop=True)
                return w0ps

            def emit_wlps():
                wlps = psa.tile([128, 36], f32, tag="pU")
                nc.tensor.matmul(wlps[:, 0:18], w3fr, rhs1[:, 0:18], start=True, stop=False)
                nc.tensor.matmul(wlps[:, 0:18], w3fn, rhs1[:, 18:36], start=False, stop=False)
                nc.tensor.matmul(wlps[:, 0:18], w3fr, rhs2[:, 0:18], start=False, stop=False)
                nc.tensor.matmul(wlps[:, 0:18], w3fn, rhs2[:, 18:36], start=False, stop=False)
                nc.tensor.matmul(wlps[:, 0:18], onesf, tl2[:, 0:18], start=False, stop=True)
                nc.tensor.matmul(wlps[:, 18:36], w3fi, rhs1[:, 0:18], start=True, stop=False)
                nc.tensor.matmul(wlps[:, 18:36], w3fr, rhs1[:, 18:36], start=False, stop=False)
                nc.tensor.matmul(wlps[:, 18:36], w3fi, rhs2[:, 0:18], start=False, stop=False)
                nc.tensor.matmul(wlps[:, 18:36], w3fr, rhs2[:, 18:36], start=False, stop=False)
                nc.tensor.matmul(wlps[:, 18:36], onesf, tl2[:, 18:36], start=False, stop=True)
                return wlps

            # ---- Phase A: F1 (plane-stacked, single-pass) + pivot-C ----
            w3r = cw3[:, 0:128]
            w3i = cw3[:, 128:256]
            w3n = cw3[:, 256:384]

            Abig = bp.tile([128, 18432], bf16, tag="Abig")
            Ubig = bp.tile([128, 18432], bf16, tag="Ubig")

            def ev_op(engine, dst, src):
                # pool/gpsimd cannot access PSUM on TRN2 hardware
                if engine == "act":
                    nc.scalar.activation(dst, src, AF.Copy)
                else:
                    nc.vector.tensor_copy(dst, src)

            def dma_q(engine):
                # ACT-produced data ships from the ACT queue (the DMA trails
                # its producer in-order, no head-of-line blocking); DVE can't
                # issue HWDGE DMAs here so its pivots ride the SP queue
                return nc.scalar if engine == "act" else nc.sync

            # Baseline-shape F1: per (si, g) 4 matmuls into a [128,1024]
            # psum tile (plane halves, 2-step accumulation); evict alternates
            # ACT/DVE and the pivot DMA ships from the evicting engine's
            # queue (ACT) or SP (for DVE) so it trails its producer.
            EVA = ["act", "dve"] * 12
            fa_pend = []  # (ab, rows, g, k2lo, nk)

            def fa_flush(idx):
                ab, rows, g, k2lo, nk = fa_pend[idx]
                eng = EVA[idx]
                stg = stp.tile([128, 1024], bf16, tag="stg")
                ev_op(eng, stg[:rows, :], ab[:rows, :])
                dma_q(eng).dma_start(
                    out=bass.AP(Abig.tensor,
                                Abig[:].offset + (16 * g) * 18432 + k2lo * 1024,
                                [[18432, 16], [1024, nk], [1, 1024]]),
                    in_=bass.AP(stg.tensor, stg[:].offset,
                                [[1024, 16 * nk], [1, 1024]]),
                )

            # ---- C*N = W0^2 * WL and the G build (defs; emitted in-loop) ----
            w0sb = tp.tile([128, 36], f32, tag="w0sb")
            wlsb = tp.tile([128, 36], f32, tag="wlsb")
            crn = tp.tile([128, 18], f32, tag="crn")
            cin = tp.tile([128, 18], f32, tag="cin")

            def emit_cn():
                ca = tp.tile([128, 18], f32, tag="ca")
                cb = tp.tile([128, 18], f32, tag="cb")
                cm1 = tp.tile([128, 18], f32, tag="cm1")
                cm2 = tp.tile([128, 18], f32, tag="cm2")
                nc.vector.tensor_mul(cm1[:], w0sb[:, 0:18], w0sb[:, 0:18])
                nc.vector.tensor_mul(cm2[:], w0sb[:, 18:36], w0sb[:, 18:36])
                nc.vector.tensor_sub(ca[:], cm1[:], cm2[:])
                nc.vector.scalar_tensor_tensor(cb[:], w0sb[:, 0:18], 2.0,
                                               w0sb[:, 18:36], AO.mult, AO.mult)
                nc.vector.tensor_mul(cm1[:], ca[:], wlsb[:, 0:18])
                nc.vector.tensor_mul(cm2[:], cb[:], wlsb[:, 18:36])
                nc.vector.tensor_sub(crn[:], cm1[:], cm2[:])
                nc.vector.tensor_mul(cm1[:], ca[:], wlsb[:, 18:36])
                nc.vector.tensor_mul(cm2[:], cb[:], wlsb[:, 0:18])
                nc.vector.tensor_add(cin[:], cm1[:], cm2[:])

            # G planes: Gr, Gi plus G2r = 2*Gr and Gn2 = -2*Gi (the factor 2
            # of P2 = 2*Xr*Xi folds here so the z_i plane is a bare product,
            # computable on Pool). Scalar-ptr ops are DVE-only; the
            # immediate-scalar doubles run on Pool.
            Gr = bp.tile([128, 2304], bf16, tag="Gr")
            Gi = bp.tile([128, 2304], bf16, tag="Gi")
            G2r = bp.tile([128, 2304], bf16, tag="G2r")
            Gn2 = bp.tile([128, 2304], bf16, tag="Gn2")
            wbr = cwiB[:, 0:2304]
            wbi = cwiB[:, 2304:4608]

            def g_piece(k2, eng):
                v = nc.vector if eng == "dve" else nc.gpsimd
                ksl = slice(k2 * 128, (k2 + 1) * 128)
                crc = crn[:, k2:k2 + 1]
                cic = cin[:, k2:k2 + 1]
                gA = gp.tile([128, 128], bf16, tag="gA")
                v.tensor_scalar(gA[:], wbi[:, ksl], cic, None, AO.mult)
                v.scalar_tensor_tensor(Gr[:, ksl], wbr[:, ksl], crc, gA[:],
                                       AO.mult, AO.subtract)
                gB = gp.tile([128, 128], bf16, tag="gB")
                v.tensor_scalar(gB[:], wbr[:, ksl], cic, None, AO.mult)
                v.scalar_tensor_tensor(Gi[:, ksl], wbi[:, ksl], crc, gB[:],
                                       AO.mult, AO.add)
                nc.gpsimd.tensor_scalar(G2r[:, ksl], Gr[:, ksl], 2.0, None, AO.mult)
                nc.gpsimd.tensor_scalar(Gn2[:, ksl], Gi[:, ksl], -2.0, None, AO.mult)

            # ---- Phase B: F3 + square + I1, software-pipelined by k2 ----
            # per-iteration: F3(k2), I1(k2-3), xc/sq/zr (ACT copy + DVE 2x),
            # z_i on Pool (bare product; factor 2 folded into G2r/Gn2),
            # ev(k2-4) alternating ACT/DVE.
            EVB = ["act", "dve"] * 9
            zt = [None] * N2
            upst = [None] * N2

            def b_f3(k2):
                ar = Abig[:, k2 * 1024 : k2 * 1024 + 512]
                ai = Abig[:, k2 * 1024 + 512 : (k2 + 1) * 1024]
                xps = psa.tile([128, 1024], f32, tag="pX")
                nc.tensor.matmul(xps[:, 0:512], w3r, ar, start=True, stop=False)
                nc.tensor.matmul(xps[:, 0:512], w3n, ai, start=False, stop=True)
                nc.tensor.matmul(xps[:, 512:1024], w3i, ar, start=True, stop=False)
                nc.tensor.matmul(xps[:, 512:1024], w3r, ai, start=False, stop=True)
                return xps

            def b_i1(k2):
                z = zt[k2]
                gsl = slice(k2 * 128, (k2 + 1) * 128)
                ups = psa.tile([128, 1024], f32, tag="pU")
                nc.tensor.matmul(ups[:, 0:512], Gr[:, gsl], z[:, 0:512], start=True, stop=False)
                nc.tensor.matmul(ups[:, 0:512], Gn2[:, gsl], z[:, 512:1024], start=False, stop=True)
                nc.tensor.matmul(ups[:, 512:1024], Gi[:, gsl], z[:, 0:512], start=True, stop=False)
                nc.tensor.matmul(ups[:, 512:1024], G2r[:, gsl], z[:, 512:1024], start=False, stop=True)
                upst[k2] = ups

            def b_sq(k2, xps):
                # one ACT pass moves both X planes to SBUF; squares and the
                # cross product then run in DVE 2x / Pool territory
                xc = xp2.tile([128, 1024], bf16, tag="xc")
                nc.scalar.activation(xc[:], xps[:], AF.Copy)
                sq = sp.tile([128, 1024], bf16, tag="sq")
                nc.vector.tensor_mul(sq[:], xc[:], xc[:])
                z = zp.tile([128, 1024], bf16, tag="z")
                nc.vector.tensor_sub(z[:, 0:512], sq[:, 0:512], sq[:, 512:1024])
                nc.gpsimd.tensor_tensor(z[:, 512:1024], xc[:, 0:512],
                                        xc[:, 512:1024], AO.mult)
                zt[k2] = z

            def b_ev(k2):
                ev_op(EVB[k2], Ubig[:, k2 * 1024 : (k2 + 1) * 1024], upst[k2][:])

            def b_iter(k2):
                xps = b_f3(k2)
                if k2 >= 2:
                    b_i1(k2 - 2)
                b_sq(k2, xps)
                if k2 >= 3:
                    b_ev(k2 - 3)
                if k2 < 6:
                    g_piece(k2 + 12, "dve")
                if k2 < 3:
                    # phase-C consts, loaded once the input storm is over
                    vs = slice(k2 * 2176, (k2 + 1) * 2176)
                    nc.sync.dma_start(out=ci2[:, vs], in_=d["ci2"][:, vs])

            cf1r = cf1[:, 0:2304]
            cf1i = cf1[:, 2304:4608]
            cf1n = cf1[:, 4608:6912]
            it = 0
            w0ps = wlps = None
            for si, (k2lo, k2hi) in enumerate(K2SPLIT):
                nk = k2hi - k2lo
                rows = nk * 16
                sbase = si * 128
                for g in range(8):
                    if si == 0 and g < 6:
                        in_chunk(g + 2)
                    csl = slice(g * F1COLS + sbase, g * F1COLS + sbase + rows)
                    xr = xt[:, g * 1024 : g * 1024 + 512]
                    xi = xt[:, g * 1024 + 512 : (g + 1) * 1024]
                    ab = psa.tile([128, 1024], f32, tag="pX" if it % 2 == 0 else "pU")
                    nc.tensor.matmul(ab[:rows, 0:512], cf1r[:, csl], xr, start=True, stop=False)
                    nc.tensor.matmul(ab[:rows, 0:512], cf1n[:, csl], xi, start=False, stop=True)
                    nc.tensor.matmul(ab[:rows, 512:1024], cf1i[:, csl], xr, start=True, stop=False)
                    nc.tensor.matmul(ab[:rows, 512:1024], cf1r[:, csl], xi, start=False, stop=True)
                    fa_pend.append((ab, rows, g, k2lo, nk))
                    if it >= 1:
                        fa_flush(it - 1)
                    if it == 1:
                        w0ps = emit_w0ps()
                    elif it == 2:
                        nc.scalar.activation(w0sb[:], w0ps[:], AF.Copy)
                    elif it == 3:
                        wlps = emit_wlps()
                    elif it == 4:
                        nc.scalar.activation(wlsb[:], wlps[:], AF.Copy)
                    elif it == 6:
                        emit_cn()
                    elif it == 8:
                        nc.gpsimd.dma_start(out=cwiB[:, 0:2304], in_=d["cwiB"][:, 0:2304])
                    elif it == 10:
                        nc.gpsimd.dma_start(out=cwiB[:, 2304:4608],
                                            in_=d["cwiB"][:, 2304:4608])
                    elif it in (12, 14, 16, 18):
                        g_piece((it - 12) // 2, "dve")
                    elif it == 20:
                        g_piece(4, "dve")
                        g_piece(5, "dve")
                        nc.sync.dma_start(out=cw3[:], in_=d["cw3"][:, :])
                    elif it in (13, 15, 17, 19, 21, 23):
                        g_piece(6 + (it - 13) // 2, "dve")
                    it += 1
            fa_flush(it - 1)

            # keep the PE p-state warm across the wdft -> F3(0) gap
            for wf in range(4):
                warm1 = psa.tile([128, 1024], f32, tag="pX" if wf % 2 == 0 else "pU")
                nc.tensor.matmul(warm1[:, 0:272], cw3f[:, 0:128], csm[:, 0:272],
                                 start=True, stop=True)
            if DEBUG_ABIG:
                nc.sync.dma_start(out=adbg[:, :], in_=Abig[:, :])

            for k2 in range(N2):
                b_iter(k2)
            for k2 in (N2 - 2, N2 - 1):
                b_i1(k2)
            for k2 in (N2 - 3, N2 - 2, N2 - 1):
                b_ev(k2)

            # ---- Phase C: pivot-D + I2 + |.| + store, pipelined by t ----
            # 19 n1-groups of 7 (last: 2). Per t: pivot-D (ACT queue, trails
            # the Ubig evicts in-order), 4 matmuls [sz*17, 512], |.|^2
            # (ACT/DVE alternating), add (DVE/Pool), sqrt (ACT), out-DMA (SP).
            ci2r = ci2[:, 0:2176]
            ci2i = ci2[:, 2176:4352]
            ci2n = ci2[:, 4352:6528]
            NT = 19

            def c_pivd(t):
                sz = 7 if t < 18 else 2
                u2 = up.tile([128, 1024], bf16, tag="u2")
                nc.gpsimd.dma_start(
                    out=bass.AP(u2.tensor, u2[:].offset,
                                [[1024, sz * 18], [1, 1024]]),
                    in_=bass.AP(Ubig.tensor, Ubig[:].offset + 7 * t * 18432,
                                [[18432, sz], [1024, 18], [1, 1024]]),
                )
                return u2

            def c_mm(t, u2):
                sz = 7 if t < 18 else 2
                rows = sz * 17
                parts = sz * 18
                csl = slice(t * 119, t * 119 + rows)
                yps = psa.tile([128, 1024], f32, tag="pX" if t % 2 == 0 else "pU")
                nc.tensor.matmul(yps[:rows, 0:512], ci2r[:parts, csl], u2[:parts, 0:512],
                                 start=True, stop=False)
                nc.tensor.matmul(yps[:rows, 0:512], ci2n[:parts, csl], u2[:parts, 512:1024],
                                 start=False, stop=True)
                nc.tensor.matmul(yps[:rows, 512:1024], ci2i[:parts, csl], u2[:parts, 0:512],
                                 start=True, stop=False)
                nc.tensor.matmul(yps[:rows, 512:1024], ci2r[:parts, csl], u2[:parts, 512:1024],
                                 start=False, stop=True)
                return yps

            def c_post(t, yps, yy):
                sz = 7 if t < 18 else 2
                rows = sz * 17
                m = sp.tile([128, 1024], bf16, tag="m")
                if t % 2 == 0 and t != 18:
                    nc.scalar.activation(m[:rows, :], yps[:rows, :], AF.Square)
                else:
                    mc = xp2.tile([128, 1024], bf16, tag="xc")
                    nc.vector.tensor_copy(mc[:rows, :], yps[:rows, :])
                    nc.vector.tensor_mul(m[:rows, :], mc[:rows, :], mc[:rows, :])
                va = nc.vector if (t % 2 == 0 or t >= 15) else nc.gpsimd
                va.tensor_tensor(yy[:rows, (t % 2) * 512 : (t % 2) * 512 + 512],
                                 m[:rows, 0:512], m[:rows, 512:1024], AO.add)

            def c_fin(p, yy, nt):
                # sqrt + store for the t-pair (2p, 2p+1); nt = tiles in pair
                rows = 119 if p < 9 else 34
                nc.scalar.activation(yy[:rows, 0:512 * nt], yy[:rows, 0:512 * nt],
                                     AF.Sqrt)
                nc.sync.dma_start(
                    out=yraw[0:rows, 2 * p * 512 : (2 * p + nt) * 512],
                    in_=yy[:rows, 0:512 * nt])  # rows<119 tail: host ignores rest

            # keep the PE p-state warm across the Ubig -> pivot-D bubble
            for wf in range(8):
                warm = psa.tile([128, 1024], f32, tag="pX" if wf % 2 == 0 else "pU")
                nc.tensor.matmul(warm[:, 0:512], w3r, Abig[:, 0:512], start=True, stop=True)
                nc.tensor.matmul(warm[:, 512:1024], w3i, Abig[:, 0:512], start=True, stop=True)
            u2_t = {0: c_pivd(0), 1: c_pivd(1), 2: c_pivd(2)}
            yps_t = {}
            yy_t = {}
            for t in range(NT):
                yps_t[t] = c_mm(t, u2_t.pop(t))
                if t + 3 < NT:
                    u2_t[t + 3] = c_pivd(t + 3)
                if t >= 1:
                    tp_ = t - 1
                    if tp_ % 2 == 0:
                        yyt = yp.tile([128, 1024], bf16, tag="yy")
                        yy_t[tp_ // 2] = yyt
                    c_post(tp_, yps_t.pop(tp_), yy_t[tp_ // 2])
                    if tp_ % 2 == 1 and tp_ >= 3:
                        c_fin(tp_ // 2 - 1, yy_t.pop(tp_ // 2 - 1), 2)
            yyt = yp.tile([128, 1024], bf16, tag="yy")
            yy_t[9] = yyt
            c_post(NT - 1, yps_t.pop(NT - 1), yy_t[9])
            c_fin(8, yy_t.pop(8), 2)
            c_fin(9, yy_t.pop(9), 1)

    nc.compile()
    return nc


_NC_CACHE = None


# ---------------- host-side orchestration ----------------
def _host_x(x_real, x_imag):
    """[Bc, 1024] f32 -> xt [128, 8192] bf16: p = 8j+n2, free = g*1024+plane*512+s."""
    out = np.empty((NCORES, 128, 8, 2, 512), BF)
    for cid in range(NCORES):
        rows = slice(cid * S, (cid + 1) * S)
        for pi, arr in enumerate((x_real, x_imag)):
            a = arr[rows].reshape(S, 8, 8, 16)          # (s, n2, g, j)
            a = a.transpose(3, 1, 2, 0)                 # (j, n2, g, s)
            out[cid, :, :, pi, :] = a.reshape(128, 8, S).astype(BF)
    return out.reshape(NCORES, 128, 8192)


def _build_wpack(w0r, w0i, wlr, wli):
    wp = np.zeros((128, 10), np.float32)
    wp[:, 0] = w0r[0:128]
    wp[:, 1] = w0i[0:128]
    wp[:, 2] = wlr[0:128]
    wp[:, 3] = wli[0:128]
    wp[:, 4] = wlr[128:256]
    wp[:, 5] = wli[128:256]
    wp[0, 6] = w0r[128]
    wp[0, 7] = w0i[128]
    wp[0, 8] = wlr[256]
    wp[0, 9] = wli[256]
    return wp


def _out_maps():
    """(rows, col_base_per_t, out_col) for valid outputs."""
    rr, cc, oo = [], [], []
    for t in range(19):
        sz = 7 if t < 18 else 2
        for jp in range(sz):
            n1 = 7 * t + jp
            for qi in range(17):
                q = qi + 1
                n = q * 128 + n1
                if CROP0 <= n < CROP0 + CLASS_NUM:
                    rr.append(jp * 17 + qi)
                    cc.append(t * 512)
                    oo.append(n - CROP0)
    return np.array(rr), np.array(cc), np.array(oo)


_OUT_R, _OUT_C, _OUT_O = _out_maps()


def kernel(**inputs):
    global _NC_CACHE
    x_real = np.ascontiguousarray(inputs["x_real"], dtype=np.float32)
    x_imag = np.ascontiguousarray(inputs["x_imag"], dtype=np.float32)
    w0r = np.ascontiguousarray(inputs["w0_real"], dtype=np.float32)
    w0i = np.ascontiguousarray(inputs["w0_imag"], dtype=np.float32)
    wlr = np.ascontiguousarray(inputs["wl_real"], dtype=np.float32)
    wli = np.ascontiguousarray(inputs["wl_imag"], dtype=np.float32)

    xts = _host_x(x_real, x_imag)
    wp = _build_wpack(w0r, w0i, wlr, wli)

    const_maps = {nm: np.ascontiguousarray(arr) for nm, arr in CONSTS.items()}
    in_maps = []
    for cid in range(NCORES):
        m = {"xt": np.ascontiguousarray(xts[cid]), "wpack": wp}
        m.update(const_maps)
        in_maps.append(m)

    if _NC_CACHE is None:
        _NC_CACHE = build_nc()
    res = run_bass_kernel_spmd(_NC_CACHE, in_maps, core_ids=list(range(NCORES)))

    out = np.empty((B, CLASS_NUM), np.float32)
    for cid in range(NCORES):
        yr = np.asarray(res.results[cid]["yraw"], dtype=np.float32)  # [119, 9728]
        # gather: out[s, oo] = yr[rr, cc + s]
        sub = yr[_OUT_R[:, None], _OUT_C[:, None] + np.arange(S)[None, :]]  # [nv, S]
        out[cid * S : (cid + 1) * S, _OUT_O] = sub.T
    return out



# revision 88
# speedup vs baseline: 1.0028x; 1.0028x over previous
"""Trainium2 Bass kernel for nn_CNN_Comp_29240137351522 (dense_cnn), v2.

Math:  y = |IFFT_N( FFT_N(x)^2 * C )|,  C = FFT_N(w0)^2 * FFT_N(wl) / N
with N = 2304 (= 128*18).  2304 >= 2303 covers the autoconv h*h exactly, and
the final circular conv aliases y[n+2304] only onto n < 255, which the center
crop [255:2303) discards, so the cropped result is exact.

Device decomposition per core (data-parallel over batch, S = 512 samples):
  n = n2*128 + n1 (n2 in [0,18), x nonzero for n2 < 8),  k = 18*k1 + k2
  F1 (contract n2, block-diag over j = n1 mod 16, twiddle folded, bf16);
      the weight-DFT, C*N and G builds are interleaved into the F1 loop
  pivot-C (DMA)   -> Abig[n1, (k2, plane, s)]; evicts alternate ACT/DVE and
      each pivot ships from its producer queue (ACT) or SP, so nothing
      head-of-line blocks the in-order DMA paths; bulky late consts ride
      the Pool SWDGE path instead of HWDGE
  F3 (contract n1, shared W128, bf16) -> X[k1, (k2, s)] in PSUM
  square: one ACT copy X->SBUF, squares/Zr on DVE 2x, Zi = Xr*Xi on Pool
      (the factor 2 is folded into the G2r/Gn2 planes of G)
  I1 (contract k1, G = C-row-scaled inverse DFT built on device, bf16)
  pivot-D (Pool SWDGE DMA) -> u2[(jp, k2), (plane, s)], 19 n1-groups of 7
  I2 (contract k2, block-diag over jp, bf16) + |.|^2 + sqrt, with sqrt and
      the store batched per pair of groups -> yraw (bf16)
Host does data movement only: batch shard, x permutation into the F1-ready
layout, packing of weight vectors, and the output unscramble.
"""

import os

import numpy as np
import ml_dtypes

DEBUG_ABIG = bool(os.environ.get("KDEBUG_ABIG"))

import concourse.bass as bass
import concourse.bacc as bacc
import concourse.mybir as mybir
from concourse.tile import TileContext
from concourse.bass_utils import run_bass_kernel_spmd

# ---------------- static problem config ----------------
B, NX = 4096, 1024
K0, KL = 129, 257
N = 2304
N1, N2 = 128, 18
NCORES = 8
S = B // NCORES              # 512 samples per core, single chunk
CROP0 = 255
CLASS_NUM = 2048
K2SPLIT = ((0, 8), (8, 16), (16, 18))     # F1 column splits (k2-major)
F1COLS = 288                               # 18*16 cols per g
JBLK = ((0, 7), (7, 14), (14, 16))        # I2 j-blocks per g
I2COLS = 272                               # 16*17 cols per g
YROWS = 119                                # max I2 out rows (7*17)

f32 = mybir.dt.float32
f32r = mybir.dt.float32r
bf16 = mybir.dt.bfloat16
AO = mybir.AluOpType
AF = mybir.ActivationFunctionType

BF = ml_dtypes.bfloat16


def _w(num, den):
    return np.exp(-2j * np.pi * np.asarray(num, np.float64) / den)


# ---------------- host-side constant arrays ----------------
def _build_consts():
    c = {}
    n1g = np.arange(N1)
    k1g = np.arange(N1)
    k2g = np.arange(N2)

    # F1 lhsT [128, 8*288]: row p = 8j + n2 ; col g*288 + sbase + k2sub*16 + j
    # value W18^{n2 k2} * W2304^{(16g+j) k2}
    f1 = np.zeros((128, 8 * F1COLS), np.complex128)
    for g in range(8):
        for (k2lo, k2hi), sbase in zip(K2SPLIT, (0, 128, 256)):
            nk = k2hi - k2lo
            for k2 in range(k2lo, k2hi):
                for j in range(16):
                    n1 = 16 * g + j
                    col = g * F1COLS + sbase + j * nk + (k2 - k2lo)
                    vals = _w(np.arange(8) * k2, N2) * _w(n1 * k2, N)
                    f1[8 * j : 8 * j + 8, col] = vals
    c["cf1"] = np.concatenate(
        [f1.real, f1.imag, -f1.imag], axis=1).astype(BF)   # [128, 3*2304]

    # F3 lhsT (shared): W128[n1,k1], bf16 + f32 copy for the weight-DFT mms
    w3 = _w(np.outer(n1g, k1g), N1)
    w3cat = np.concatenate([w3.real, w3.imag, -w3.imag], axis=1)
    c["cw3"] = w3cat.astype(BF)                            # [128, 384]
    c["cw3f"] = w3cat.astype(np.float32)                   # [128, 384]

    # inverse-DFT base tiled over k2, divided by N (folds the 1/N of C):
    # cwiB[:, v*2304 + k2*128 + p] = {Re,Im}(W128^{-k1 p}) / N
    wi = _w(-np.outer(k1g, n1g), N1) / N
    blk = np.concatenate([np.tile(wi.real, (1, N2)), np.tile(wi.imag, (1, N2))], axis=1)
    c["cwiB"] = blk.astype(BF)                             # [128, 2*2304]

    # I2 lhsT [128, 3*2176]: 19 n1-groups (18 of size 7 + 1 of size 2).
    # Per group t: partition p = jp*18 + k2, col TB[t] + jp*17 + (q-1);
    # value W18^{-q k2} * W2304^{-(7t+jp) k2}, q in [1,18)
    i2 = np.zeros((128, 2176), np.complex128)
    qg = np.arange(1, 18)
    for t in range(19):
        sz = 7 if t < 18 else 2
        for jp in range(sz):
            n1 = 7 * t + jp
            blkv = _w(-np.outer(k2g, qg), N2) * _w(-n1 * k2g, N)[:, None]
            rows = slice(jp * 18, jp * 18 + 18)
            cols = slice(t * 119 + jp * 17, t * 119 + (jp + 1) * 17)
            i2[rows, cols] = blkv
    c["ci2"] = np.concatenate(
        [i2.real, i2.imag, -i2.imag], axis=1).astype(BF)   # [128, 3*2176]

    # weight-DFT rhs constants (f32), packed into one [128, 272] tensor:
    # cols 0:18 ct1r | 18:36 ct1i | 36:54 ct2r | 54:72 ct2i
    # row0 cols 72:90 te1r | 90:108 te1i | 108:126 te2r | 126:144 te2i
    # row0 cols 144:272 ones (128)
    nh = np.arange(128)
    sm = np.zeros((128, 272), np.float32)
    t1 = _w(np.outer(nh, k2g), N)
    sm[:, 0:18] = t1.real
    sm[:, 18:36] = t1.imag
    t2 = _w(np.outer(nh, k2g), N) * _w(k2g, N2)[None, :]
    sm[:, 36:54] = t2.real
    sm[:, 54:72] = t2.imag
    te1 = _w(k2g, N2)
    sm[0, 72:90] = te1.real
    sm[0, 90:108] = te1.imag
    te2 = _w(k2g, 9)
    sm[0, 108:126] = te2.real
    sm[0, 126:144] = te2.imag
    sm[0, 144:272] = 1.0
    c["csm"] = sm

    return c


CONSTS = _build_consts()


# ---------------- bass kernel builder ----------------
def build_nc():
    nc = bacc.Bacc("TRN2", target_bir_lowering=False, debug=False, num_devices=NCORES)

    d = {}
    d["xt"] = nc.dram_tensor("xt", [128, 8192], bf16, kind="ExternalInput")
    d["wpack"] = nc.dram_tensor("wpack", [128, 10], f32, kind="ExternalInput")
    cdt = {"cw3f": f32, "csm": f32}
    for nm, arr in CONSTS.items():
        d[nm] = nc.dram_tensor(nm, list(arr.shape), cdt.get(nm, bf16), kind="ExternalInput")
    yraw = nc.dram_tensor("yraw", [YROWS, 19 * 512], bf16, kind="ExternalOutput")
    adbg = (nc.dram_tensor("adbg", [128, 18432], bf16, kind="ExternalOutput")
            if DEBUG_ABIG else None)

    with TileContext(nc) as tc:
        with (
            tc.tile_pool(name="cp", bufs=1) as cp,          # persistent consts
            tc.tile_pool(name="bp", bufs=1) as bp,          # Abig / Ubig / G
            tc.tile_pool(name="sp", bufs=2) as sp,          # rotating stage tiles
            tc.tile_pool(name="gp", bufs=2) as gp,          # G-build temporaries
            tc.tile_pool(name="stp", bufs=6) as stp,        # pivot-C staging
            tc.tile_pool(name="xp2", bufs=2) as xp2,        # xi/y copies
            tc.tile_pool(name="up", bufs=3) as up,          # u2 tiles
            tc.tile_pool(name="yp", bufs=2) as yp,          # yy tiles
            tc.tile_pool(name="zp", bufs=4) as zp,          # z tiles
            tc.tile_pool(name="tp", bufs=1) as tp,          # small f32 tmps
            tc.tile_pool(name="psa", bufs=2, space="PSUM") as psa,
        ):
            # ---- const + input DMAs (sync engine; just-in-time ordering:
            # inputs are chunked per-2u and interleaved with the pivot DMAs
            # inside the F1 loop so pivots aren't queued behind the whole
            # input storm on the in-order SP/DMA path) ----
            cf1 = cp.tile([128, 3 * 2304], bf16, tag="cf1")
            xt = cp.tile([128, 8192], bf16, tag="xt")

            def in_chunk(q):               # xt cols for g = q (old layout)
                gs = slice(q * 1024, (q + 1) * 1024)
                nc.sync.dma_start(out=xt[:, gs], in_=d["xt"][:, gs])

            nc.sync.dma_start(out=cf1[:, 0:2304], in_=d["cf1"][:, 0:2304])
            in_chunk(0)
            wpk = cp.tile([128, 10], f32, tag="wpack")
            nc.sync.dma_start(out=wpk[:], in_=d["wpack"][:, :])
            csm = cp.tile([128, 272], f32, tag="csm")
            nc.sync.dma_start(out=csm[:], in_=d["csm"][:, :])
            nc.sync.dma_start(out=cf1[:, 4608:6912], in_=d["cf1"][:, 4608:6912])
            in_chunk(1)
            nc.sync.dma_start(out=cf1[:, 2304:4608], in_=d["cf1"][:, 2304:4608])
            cw3f = cp.tile([128, 384], f32, tag="cw3f")
            nc.sync.dma_start(out=cw3f[:], in_=d["cw3f"][:, :])
            cw3 = cp.tile([128, 384], bf16, tag="cw3")
            cwiB = cp.tile([128, 2 * 2304], bf16, tag="cwiB")
            ci2 = cp.tile([128, 3 * 2176], bf16, tag="ci2")

            # ---- weight DFT rhs prep (DVE; needs only wpack/csm) ----
            def cplx_rhs(rows, tr, ti, cr, ci, outr, outi):
                # (cr + i ci) * (tr + i ti); cr/ci are [rows,1] scalar APs
                t = tp.tile([128, 18], f32, tag="wtmp")
                nc.vector.tensor_scalar(t[:rows, :], ti, ci, None, AO.mult)
                nc.vector.scalar_tensor_tensor(outr, tr, cr, t[:rows, :], AO.mult, AO.subtract)
                t2 = tp.tile([128, 18], f32, tag="wtmp2")
                nc.vector.tensor_scalar(t2[:rows, :], tr, ci, None, AO.mult)
                nc.vector.scalar_tensor_tensor(outi, ti, cr, t2[:rows, :], AO.mult, AO.add)

            rhs0 = tp.tile([128, 36], f32, tag="rhs0")
            cplx_rhs(128, csm[:, 0:18], csm[:, 18:36], wpk[:, 0:1], wpk[:, 1:2],
                     rhs0[:, 0:18], rhs0[:, 18:36])
            tl0 = tp.tile([1, 36], f32, tag="tl0")
            cplx_rhs(1, csm[0:1, 72:90], csm[0:1, 90:108], wpk[0:1, 6:7], wpk[0:1, 7:8],
                     tl0[:, 0:18], tl0[:, 18:36])
            rhs1 = tp.tile([128, 36], f32, tag="rhs1")
            cplx_rhs(128, csm[:, 0:18], csm[:, 18:36], wpk[:, 2:3], wpk[:, 3:4],
                     rhs1[:, 0:18], rhs1[:, 18:36])
            rhs2 = tp.tile([128, 36], f32, tag="rhs2")
            cplx_rhs(128, csm[:, 36:54], csm[:, 54:72], wpk[:, 4:5], wpk[:, 5:6],
                     rhs2[:, 0:18], rhs2[:, 18:36])
            tl2 = tp.tile([1, 36], f32, tag="tl2")
            cplx_rhs(1, csm[0:1, 108:126], csm[0:1, 126:144], wpk[0:1, 8:9], wpk[0:1, 9:10],
                     tl2[:, 0:18], tl2[:, 18:36])

            w3fr = cw3f[:, 0:128]
            w3fi = cw3f[:, 128:256]
            w3fn = cw3f[:, 256:384]
            onesf = csm[0:1, 144:272]

            def emit_w0ps():
                w0ps = psa.tile([128, 36], f32, tag="pX")
                nc.tensor.matmul(w0ps[:, 0:18], w3fr, rhs0[:, 0:18], start=True, stop=False)
                nc.tensor.matmul(w0ps[:, 0:18], w3fn, rhs0[:, 18:36], start=False, stop=False)
                nc.tensor.matmul(w0ps[:, 0:18], onesf, tl0[:, 0:18], start=False, stop=True)
                nc.tensor.matmul(w0ps[:, 18:36], w3fi, rhs0[:, 0:18], start=True, stop=False)
                nc.tensor.matmul(w0ps[:, 18:36], w3fr, rhs0[:, 18:36], start=False, stop=False)
                nc.tensor.matmul(w0ps[:, 18:36], onesf, tl0[:, 18:36], start=False, stop=True)
                return w0ps

            def emit_wlps():
                wlps = psa.tile([128, 36], f32, tag="pU")
                nc.tensor.matmul(wlps[:, 0:18], w3fr, rhs1[:, 0:18], start=True, stop=False)
                nc.tensor.matmul(wlps[:, 0:18], w3fn, rhs1[:, 18:36], start=False, stop=False)
                nc.tensor.matmul(wlps[:, 0:18], w3fr, rhs2[:, 0:18], start=False, stop=False)
                nc.tensor.matmul(wlps[:, 0:18], w3fn, rhs2[:, 18:36], start=False, stop=False)
                nc.tensor.matmul(wlps[:, 0:18], onesf, tl2[:, 0:18], start=False, stop=True)
                nc.tensor.matmul(wlps[:, 18:36], w3fi, rhs1[:, 0:18], start=True, stop=False)
                nc.tensor.matmul(wlps[:, 18:36], w3fr, rhs1[:, 18:36], start=False, stop=False)
                nc.tensor.matmul(wlps[:, 18:36], w3fi, rhs2[:, 0:18], start=False, stop=False)
                nc.tensor.matmul(wlps[:, 18:36], w3fr, rhs2[:, 18:36], start=False, stop=False)
                nc.tensor.matmul(wlps[:, 18:36], onesf, tl2[:, 18:36], start=False, stop=True)
                return wlps

            # ---- Phase A: F1 (plane-stacked, single-pass) + pivot-C ----
            w3r = cw3[:, 0:128]
            w3i = cw3[:, 128:256]
            w3n = cw3[:, 256:384]

            Abig = bp.tile([128, 18432], bf16, tag="Abig")
            Ubig = bp.tile([128, 18432], bf16, tag="Ubig")

            def ev_op(engine, dst, src):
                # pool/gpsimd cannot access PSUM on TRN2 hardware
                if engine == "act":
                    nc.scalar.activation(dst, src, AF.Copy)
                else:
                    nc.vector.tensor_copy(dst, src)

            def dma_q(engine):
                # ACT-produced data ships from the ACT queue (the DMA trails
                # its producer in-order, no head-of-line blocking); DVE can't
                # issue HWDGE DMAs here so its pivots ride the SP queue
                return nc.scalar if engine == "act" else nc.sync

            # Baseline-shape F1: per (si, g) 4 matmuls into a [128,1024]
            # psum tile (plane halves, 2-step accumulation); evict alternates
            # ACT/DVE and the pivot DMA ships from the evicting engine's
            # queue (ACT) or SP (for DVE) so it trails its producer.
            EVA = ["act", "dve"] * 12
            fa_pend = []  # (ab, rows, g, k2lo, nk)

            def fa_flush(idx):
                ab, rows, g, k2lo, nk = fa_pend[idx]
                eng = EVA[idx]
                stg = stp.tile([128, 1024], bf16, tag="stg")
                ev_op(eng, stg[:rows, :], ab[:rows, :])
                dma_q(eng).dma_start(
                    out=bass.AP(Abig.tensor,
                                Abig[:].offset + (16 * g) * 18432 + k2lo * 1024,
                                [[18432, 16], [1024, nk], [1, 1024]]),
                    in_=bass.AP(stg.tensor, stg[:].offset,
                                [[1024, 16 * nk], [1, 1024]]),
                )

            # ---- C*N = W0^2 * WL and the G build (defs; emitted in-loop) ----
            w0sb = tp.tile([128, 36], f32, tag="w0sb")
            wlsb = tp.tile([128, 36], f32, tag="wlsb")
            crn = tp.tile([128, 18], f32, tag="crn")
            cin = tp.tile([128, 18], f32, tag="cin")

            def emit_cn():
                ca = tp.tile([128, 18], f32, tag="ca")
                cb = tp.tile([128, 18], f32, tag="cb")
                cm1 = tp.tile([128, 18], f32, tag="cm1")
                cm2 = tp.tile([128, 18], f32, tag="cm2")
                nc.vector.tensor_mul(cm1[:], w0sb[:, 0:18], w0sb[:, 0:18])
                nc.vector.tensor_mul(cm2[:], w0sb[:, 18:36], w0sb[:, 18:36])
                nc.vector.tensor_sub(ca[:], cm1[:], cm2[:])
                nc.vector.scalar_tensor_tensor(cb[:], w0sb[:, 0:18], 2.0,
                                               w0sb[:, 18:36], AO.mult, AO.mult)
                nc.vector.tensor_mul(cm1[:], ca[:], wlsb[:, 0:18])
                nc.vector.tensor_mul(cm2[:], cb[:], wlsb[:, 18:36])
                nc.vector.tensor_sub(crn[:], cm1[:], cm2[:])
                nc.vector.tensor_mul(cm1[:], ca[:], wlsb[:, 18:36])
                nc.vector.tensor_mul(cm2[:], cb[:], wlsb[:, 0:18])
                nc.vector.tensor_add(cin[:], cm1[:], cm2[:])

            # G planes: Gr, Gi plus G2r = 2*Gr and Gn2 = -2*Gi (the factor 2
            # of P2 = 2*Xr*Xi folds here so the z_i plane is a bare product,
            # computable on Pool). Scalar-ptr ops are DVE-only; the
            # immediate-scalar doubles run on Pool.
            Gr = bp.tile([128, 2304], bf16, tag="Gr")
            Gi = bp.tile([128, 2304], bf16, tag="Gi")
            G2r = bp.tile([128, 2304], bf16, tag="G2r")
            Gn2 = bp.tile([128, 2304], bf16, tag="Gn2")
            wbr = cwiB[:, 0:2304]
            wbi = cwiB[:, 2304:4608]

            def g_piece(k2, eng):
                v = nc.vector if eng == "dve" else nc.gpsimd
                ksl = slice(k2 * 128, (k2 + 1) * 128)
                crc = crn[:, k2:k2 + 1]
                cic = cin[:, k2:k2 + 1]
                gA = gp.tile([128, 128], bf16, tag="gA")
                v.tensor_scalar(gA[:], wbi[:, ksl], cic, None, AO.mult)
                v.scalar_tensor_tensor(Gr[:, ksl], wbr[:, ksl], crc, gA[:],
                                       AO.mult, AO.subtract)
                gB = gp.tile([128, 128], bf16, tag="gB")
                v.tensor_scalar(gB[:], wbr[:, ksl], cic, None, AO.mult)
                v.scalar_tensor_tensor(Gi[:, ksl], wbi[:, ksl], crc, gB[:],
                                       AO.mult, AO.add)
                nc.gpsimd.tensor_scalar(G2r[:, ksl], Gr[:, ksl], 2.0, None, AO.mult)
                nc.gpsimd.tensor_scalar(Gn2[:, ksl], Gi[:, ksl], -2.0, None, AO.mult)

            # ---- Phase B: F3 + square + I1, software-pipelined by k2 ----
            # per-iteration: F3(k2), I1(k2-3), xc/sq/zr (ACT copy + DVE 2x),
            # z_i on Pool (bare product; factor 2 folded into G2r/Gn2),
            # ev(k2-4) alternating ACT/DVE.
            EVB = ["act", "dve"] * 9
            zt = [None] * N2
            upst = [None] * N2

            def b_f3(k2):
                ar = Abig[:, k2 * 1024 : k2 * 1024 + 512]
                ai = Abig[:, k2 * 1024 + 512 : (k2 + 1) * 1024]
                xps = psa.tile([128, 1024], f32, tag="pX")
                nc.tensor.matmul(xps[:, 0:512], w3r, ar, start=True, stop=False)
                nc.tensor.matmul(xps[:, 0:512], w3n, ai, start=False, stop=True)
                nc.tensor.matmul(xps[:, 512:1024], w3i, ar, start=True, stop=False)
                nc.tensor.matmul(xps[:, 512:1024], w3r, ai, start=False, stop=True)
                return xps

            def b_i1(k2):
                z = zt[k2]
                gsl = slice(k2 * 128, (k2 + 1) * 128)
                ups = psa.tile([128, 1024], f32, tag="pU")
                nc.tensor.matmul(ups[:, 0:512], Gr[:, gsl], z[:, 0:512], start=True, stop=False)
                nc.tensor.matmul(ups[:, 0:512], Gn2[:, gsl], z[:, 512:1024], start=False, stop=True)
                nc.tensor.matmul(ups[:, 512:1024], Gi[:, gsl], z[:, 0:512], start=True, stop=False)
                nc.tensor.matmul(ups[:, 512:1024], G2r[:, gsl], z[:, 512:1024], start=False, stop=True)
                upst[k2] = ups

            def b_sq(k2, xps):
                # one ACT pass moves both X planes to SBUF; squares and the
                # cross product then run in DVE 2x / Pool territory
                xc = xp2.tile([128, 1024], bf16, tag="xc")
                nc.scalar.activation(xc[:], xps[:], AF.Copy)
                sq = sp.tile([128, 1024], bf16, tag="sq")
                nc.vector.tensor_mul(sq[:], xc[:], xc[:])
                z = zp.tile([128, 1024], bf16, tag="z")
                nc.vector.tensor_sub(z[:, 0:512], sq[:, 0:512], sq[:, 512:1024])
                nc.gpsimd.tensor_tensor(z[:, 512:1024], xc[:, 0:512],
                                        xc[:, 512:1024], AO.mult)
                zt[k2] = z

            def b_ev(k2):
                ev_op(EVB[k2], Ubig[:, k2 * 1024 : (k2 + 1) * 1024], upst[k2][:])

            def b_iter(k2):
                xps = b_f3(k2)
                if k2 >= 3:
                    b_i1(k2 - 3)
                b_sq(k2, xps)
                if k2 >= 4:
                    b_ev(k2 - 4)
                if k2 < 6:
                    g_piece(k2 + 12, "dve")
                if k2 < 3:
                    # phase-C consts, loaded once the input storm is over
                    vs = slice(k2 * 2176, (k2 + 1) * 2176)
                    nc.sync.dma_start(out=ci2[:, vs], in_=d["ci2"][:, vs])

            cf1r = cf1[:, 0:2304]
            cf1i = cf1[:, 2304:4608]
            cf1n = cf1[:, 4608:6912]
            it = 0
            w0ps = wlps = None
            for si, (k2lo, k2hi) in enumerate(K2SPLIT):
                nk = k2hi - k2lo
                rows = nk * 16
                sbase = si * 128
                for g in range(8):
                    if si == 0 and g < 6:
                        in_chunk(g + 2)
                    csl = slice(g * F1COLS + sbase, g * F1COLS + sbase + rows)
                    xr = xt[:, g * 1024 : g * 1024 + 512]
                    xi = xt[:, g * 1024 + 512 : (g + 1) * 1024]
                    ab = psa.tile([128, 1024], f32, tag="pX" if it % 2 == 0 else "pU")
                    nc.tensor.matmul(ab[:rows, 0:512], cf1r[:, csl], xr, start=True, stop=False)
                    nc.tensor.matmul(ab[:rows, 0:512], cf1n[:, csl], xi, start=False, stop=True)
                    nc.tensor.matmul(ab[:rows, 512:1024], cf1i[:, csl], xr, start=True, stop=False)
                    nc.tensor.matmul(ab[:rows, 512:1024], cf1r[:, csl], xi, start=False, stop=True)
                    fa_pend.append((ab, rows, g, k2lo, nk))
                    if it >= 1:
                        fa_flush(it - 1)
                    if it == 1:
                        w0ps = emit_w0ps()
                    elif it == 2:
                        nc.scalar.activation(w0sb[:], w0ps[:], AF.Copy)
                    elif it == 3:
                        wlps = emit_wlps()
                    elif it == 4:
                        nc.scalar.activation(wlsb[:], wlps[:], AF.Copy)
                    elif it == 6:
                        emit_cn()
                    elif it == 8:
                        nc.gpsimd.dma_start(out=cwiB[:, 0:2304], in_=d["cwiB"][:, 0:2304])
                    elif it == 10:
                        nc.gpsimd.dma_start(out=cwiB[:, 2304:4608],
                                            in_=d["cwiB"][:, 2304:4608])
                    elif it in (12, 14, 16, 18):
                        g_piece((it - 12) // 2, "dve")
                    elif it == 20:
                        g_piece(4, "dve")
                        g_piece(5, "dve")
                        nc.sync.dma_start(out=cw3[:], in_=d["cw3"][:, :])
                    elif it in (13, 15, 17, 19, 21, 23):
                        g_piece(6 + (it - 13) // 2, "dve")
                    it += 1
            fa_flush(it - 1)

            # keep the PE p-state warm across the wdft -> F3(0) gap
            for wf in range(4):
                warm1 = psa.tile([128, 1024], f32, tag="pX" if wf % 2 == 0 else "pU")
                nc.tensor.matmul(warm1[:, 0:272], cw3f[:, 0:128], csm[:, 0:272],
                                 start=True, stop=True)
            if DEBUG_ABIG:
                nc.sync.dma_start(out=adbg[:, :], in_=Abig[:, :])

            for k2 in range(N2):
                b_iter(k2)
            for k2 in (N2 - 3, N2 - 2, N2 - 1):
                b_i1(k2)
            for k2 in (N2 - 4, N2 - 3, N2 - 2, N2 - 1):
                b_ev(k2)

            # ---- Phase C: pivot-D + I2 + |.| + store, pipelined by t ----
            # 19 n1-groups of 7 (last: 2). Per t: pivot-D (ACT queue, trails
            # the Ubig evicts in-order), 4 matmuls [sz*17, 512], |.|^2
            # (ACT/DVE alternating), add (DVE/Pool), sqrt (ACT), out-DMA (SP).
            ci2r = ci2[:, 0:2176]
            ci2i = ci2[:, 2176:4352]
            ci2n = ci2[:, 4352:6528]
            NT = 19

            def c_pivd(t):
                sz = 7 if t < 18 else 2
                u2 = up.tile([128, 1024], bf16, tag="u2")
                nc.gpsimd.dma_start(
                    out=bass.AP(u2.tensor, u2[:].offset,
                                [[1024, sz * 18], [1, 1024]]),
                    in_=bass.AP(Ubig.tensor, Ubig[:].offset + 7 * t * 18432,
                                [[18432, sz], [1024, 18], [1, 1024]]),
                )
                return u2

            def c_mm(t, u2):
                sz = 7 if t < 18 else 2
                rows = sz * 17
                parts = sz * 18
                csl = slice(t * 119, t * 119 + rows)
                yps = psa.tile([128, 1024], f32, tag="pX" if t % 2 == 0 else "pU")
                nc.tensor.matmul(yps[:rows, 0:512], ci2r[:parts, csl], u2[:parts, 0:512],
                                 start=True, stop=False)
                nc.tensor.matmul(yps[:rows, 0:512], ci2n[:parts, csl], u2[:parts, 512:1024],
                                 start=False, stop=True)
                nc.tensor.matmul(yps[:rows, 512:1024], ci2i[:parts, csl], u2[:parts, 0:512],
                                 start=True, stop=False)
                nc.tensor.matmul(yps[:rows, 512:1024], ci2r[:parts, csl], u2[:parts, 512:1024],
                                 start=False, stop=True)
                return yps

            def c_post(t, yps, yy):
                sz = 7 if t < 18 else 2
                rows = sz * 17
                m = sp.tile([128, 1024], bf16, tag="m")
                if t % 2 == 0:
                    nc.scalar.activation(m[:rows, :], yps[:rows, :], AF.Square)
                else:
                    mc = xp2.tile([128, 1024], bf16, tag="xc")
                    nc.vector.tensor_copy(mc[:rows, :], yps[:rows, :])
                    nc.vector.tensor_mul(m[:rows, :], mc[:rows, :], mc[:rows, :])
                va = nc.vector if (t % 2 == 0 or t >= 15) else nc.gpsimd
                va.tensor_tensor(yy[:rows, (t % 2) * 512 : (t % 2) * 512 + 512],
                                 m[:rows, 0:512], m[:rows, 512:1024], AO.add)

            def c_fin(p, yy, nt):
                # sqrt + store for the t-pair (2p, 2p+1); nt = tiles in pair
                rows = 119 if p < 9 else 34
                nc.scalar.activation(yy[:rows, 0:512 * nt], yy[:rows, 0:512 * nt],
                                     AF.Sqrt)
                nc.sync.dma_start(
                    out=yraw[0:rows, 2 * p * 512 : (2 * p + nt) * 512],
                    in_=yy[:rows, 0:512 * nt])  # rows<119 tail: host ignores rest

            # keep the PE p-state warm across the Ubig -> pivot-D bubble
            for wf in range(8):
                warm = psa.tile([128, 1024], f32, tag="pX" if wf % 2 == 0 else "pU")
                nc.tensor.matmul(warm[:, 0:512], w3r, Abig[:, 0:512], start=True, stop=True)
                nc.tensor.matmul(warm[:, 512:1024], w3i, Abig[:, 0:512], start=True, stop=True)
            u2_t = {0: c_pivd(0), 1: c_pivd(1), 2: c_pivd(2)}
            yps_t = {}
            yy_t = {}
            for t in range(NT):
                yps_t[t] = c_mm(t, u2_t.pop(t))
                if t + 3 < NT:
                    u2_t[t + 3] = c_pivd(t + 3)
                if t >= 1:
                    tp_ = t - 1
                    if tp_ % 2 == 0:
                        yyt = yp.tile([128, 1024], bf16, tag="yy")
                        yy_t[tp_ // 2] = yyt
                    c_post(tp_, yps_t.pop(tp_), yy_t[tp_ // 2])
                    if tp_ % 2 == 1 and tp_ >= 3:
                        c_fin(tp_ // 2 - 1, yy_t.pop(tp_ // 2 - 1), 2)
            yyt = yp.tile([128, 1024], bf16, tag="yy")
            yy_t[9] = yyt
            c_post(NT - 1, yps_t.pop(NT - 1), yy_t[9])
            c_fin(8, yy_t.pop(8), 2)
            c_fin(9, yy_t.pop(9), 1)

    nc.compile()
    return nc


_NC_CACHE = None


# ---------------- host-side orchestration ----------------
def _host_x(x_real, x_imag):
    """[Bc, 1024] f32 -> xt [128, 8192] bf16: p = 8j+n2, free = g*1024+plane*512+s."""
    out = np.empty((NCORES, 128, 8, 2, 512), BF)
    for cid in range(NCORES):
        rows = slice(cid * S, (cid + 1) * S)
        for pi, arr in enumerate((x_real, x_imag)):
            a = arr[rows].reshape(S, 8, 8, 16)          # (s, n2, g, j)
            a = a.transpose(3, 1, 2, 0)                 # (j, n2, g, s)
            out[cid, :, :, pi, :] = a.reshape(128, 8, S).astype(BF)
    return out.reshape(NCORES, 128, 8192)


def _build_wpack(w0r, w0i, wlr, wli):
    wp = np.zeros((128, 10), np.float32)
    wp[:, 0] = w0r[0:128]
    wp[:, 1] = w0i[0:128]
    wp[:, 2] = wlr[0:128]
    wp[:, 3] = wli[0:128]
    wp[:, 4] = wlr[128:256]
    wp[:, 5] = wli[128:256]
    wp[0, 6] = w0r[128]
    wp[0, 7] = w0i[128]
    wp[0, 8] = wlr[256]
    wp[0, 9] = wli[256]
    return wp


def _out_maps():
    """(rows, col_base_per_t, out_col) for valid outputs."""
    rr, cc, oo = [], [], []
    for t in range(19):
        sz = 7 if t < 18 else 2
        for jp in range(sz):
            n1 = 7 * t + jp
            for qi in range(17):
                q = qi + 1
                n = q * 128 + n1
                if CROP0 <= n < CROP0 + CLASS_NUM:
                    rr.append(jp * 17 + qi)
                    cc.append(t * 512)
                    oo.append(n - CROP0)
    return np.array(rr), np.array(cc), np.array(oo)


_OUT_R, _OUT_C, _OUT_O = _out_maps()


def kernel(**inputs):
    global _NC_CACHE
    x_real = np.ascontiguousarray(inputs["x_real"], dtype=np.float32)
    x_imag = np.ascontiguousarray(inputs["x_imag"], dtype=np.float32)
    w0r = np.ascontiguousarray(inputs["w0_real"], dtype=np.float32)
    w0i = np.ascontiguousarray(inputs["w0_imag"], dtype=np.float32)
    wlr = np.ascontiguousarray(inputs["wl_real"], dtype=np.float32)
    wli = np.ascontiguousarray(inputs["wl_imag"], dtype=np.float32)

    xts = _host_x(x_real, x_imag)
    wp = _build_wpack(w0r, w0i, wlr, wli)

    const_maps = {nm: np.ascontiguousarray(arr) for nm, arr in CONSTS.items()}
    in_maps = []
    for cid in range(NCORES):
        m = {"xt": np.ascontiguousarray(xts[cid]), "wpack": wp}
        m.update(const_maps)
        in_maps.append(m)

    if _NC_CACHE is None:
        _NC_CACHE = build_nc()
    res = run_bass_kernel_spmd(_NC_CACHE, in_maps, core_ids=list(range(NCORES)))

    out = np.empty((B, CLASS_NUM), np.float32)
    for cid in range(NCORES):
        yr = np.asarray(res.results[cid]["yraw"], dtype=np.float32)  # [119, 9728]
        # gather: out[s, oo] = yr[rr, cc + s]
        sub = yr[_OUT_R[:, None], _OUT_C[:, None] + np.arange(S)[None, :]]  # [nv, S]
        out[cid * S : (cid + 1) * S, _OUT_O] = sub.T
    return out

